# revision 49
# baseline (speedup 1.0000x reference)
"""Causal attention (B=2, T=2048, E=1024, H=16, D=64) on 8 TRN2 NeuronCores.

Sharding: core c handles batch b = c//4 and local head group hg = c%4
(4 heads, 256 head-dims).  Data parallel over batch, tensor parallel over
heads; the output projection is row-parallel, so each core returns a
partial [T, E] output and the host sums the 4 partials per batch.

Bias algebra (host-side): softmax is over the query axis j, so the
q-bias terms k_i.bq + bk.bq are constant per softmax row and cancel;
only bk survives (applied to k on device).  Softmax rows sum to 1, so
the v-bias contributes exactly bv @ Wp.T to the output; together with
bp it is added on the HOST after the partial-sum gather.  The device
computes pure matmuls + one bk add.

Device plan (per core, all-bf16 matmuls with fp32 PSUM accumulation):
  DMA: three rings (sync HWDGE, scalar HWDGE, gpsimd SWDGE) stream
    concurrently from shared HBM in demand order.  xt is loaded as 8
    grouped 0.5 MiB tiles [P, 4ec, 512cols] (column-quarter major =
    demand order); q/k weights are hc-major so each head-chunk is one
    contiguous 0.25 MiB transfer.  The first exp batch transitively
    needs only {wq/wk hc0, xt quarter 0} — those lead their rings.
  q/k/v production: 8-matmul full-E accumulation chains through a
    2-slot PSUM pool; the first two chain readouts run on the
    still-idle ScalarE (Copy / Identity+bk-bias activations), later
    ones on VectorE.  v tiles are [ones(64) | x@Wv (64)] per head so
    the softmax denominator falls out of the PV matmul.
  attention: decoupled S/PV batches per (head-pair, i-block).
    S(p,ib): per j-tile, 2 concurrent K=64 score MMs (row groups 0-63 /
    64-127), exp on ScalarE (scale=1/8, no max subtraction — scores are
    ~N(0,1)) into SBUF-staged pt tiles, single mask multiply over both
    heads on block-diagonal tiles (one [128,2,512] lower-tri pattern;
    diagonal tile idx reads the [0:512-128*idx] column prefix).
    All S batches are pulled as far forward as data and pt-slot
    rotation allow (pt pool: 4 slots for j-tiles 0-3, 3 slots for the
    rest) so the ScalarE exp stream — the attention pacer — runs
    compact and early.  PV(p,ib) accumulates v over j from the staged
    pt tiles (never waiting on its own tile's exp); PV, the remaining
    q/k chains, v waves and the projection are the PE back-fill that
    keeps HAM warm.
  proj: per t-tile, 2 accumulating MMs + PSUM->SBUF bf16 copy (Vector
    early, ScalarE for the last tiles once the exp stream has
    drained), out DMA on the sync ring.
"""

import ml_dtypes
import numpy as np

import concourse.bass as bass
import concourse.tile as tile
from concourse import bacc, mybir
from concourse.bass_utils import run_bass_kernel_spmd

B, T, E = 2, 2048, 1024
H, D = 16, 64
NCORES = 8
GROUPS = 4              # cores per batch (tensor parallel over heads)
HL = H // GROUPS        # 4 local heads per core
HDL = HL * D            # 256 local head dims
P = 128
TQ = 512                # i-block (free dim of score tiles)
JB = 128                # j-block (partition dim of score tiles)
N_TB = T // TQ          # 4
N_EC = E // P           # 8
N_TC = T // P           # 16
HALF = T // 2           # 1024 (xt half-tile width)

F32 = mybir.dt.float32
BF16 = mybir.dt.bfloat16
AF = mybir.ActivationFunctionType


def _build_nc():
    nc = bacc.Bacc("TRN2", target_bir_lowering=False, debug=False)
    xt = nc.dram_tensor("xt", [E, T], BF16, kind="ExternalInput").ap()
    # hc-major q/k weight layout: [:, hc] is one contiguous 0.25 MiB DMA
    wqt = nc.dram_tensor("wqt", [P, 2, N_EC, P], BF16, kind="ExternalInput").ap()
    wkt = nc.dram_tensor("wkt", [P, 2, N_EC, P], BF16, kind="ExternalInput").ap()
    wvt = nc.dram_tensor("wvt", [P, N_EC, HDL], BF16, kind="ExternalInput").ap()
    wpt = nc.dram_tensor("wpt", [P, 2, E], BF16, kind="ExternalInput").ap()
    bkv = nc.dram_tensor("bkv", [HDL], F32, kind="ExternalInput").ap()
    maskd = nc.dram_tensor("mask", [JB, 2, TQ], BF16,
                           kind="ExternalInput").ap()
    onesv = nc.dram_tensor("onesv", [HDL], BF16, kind="ExternalInput").ap()
    out = nc.dram_tensor("out", [T, E], BF16, kind="ExternalOutput").ap()

    with tile.TileContext(nc) as tc:
        with (
            tc.tile_pool(name="big", bufs=1) as big,
            tc.tile_pool(name="ptp", bufs=3) as ptp,
            tc.tile_pool(name="ptp4", bufs=4) as ptp4,
            tc.tile_pool(name="work", bufs=2) as work,
            tc.tile_pool(name="outp", bufs=3) as outp,
        ):
            # ---------------- DMA posts (order = transfer order per ring) ---
            # The first exp transitively needs only q/k(hc0) weights + the
            # tb0 columns of xt, so xt is posted as [P,512] quarters and
            # wq/wk as hc slices, interleaved across all three rings in
            # demand order (rings share HBM bandwidth; per-ring streams are
            # serial, so each demand wave is split across rings):
            #   wave 0: wq/wk hc0, xt q0 (tb0)  -> S(0,0)
            #   wave 1: wv, xt q1 (tb1)         -> v chains, S(0,1)
            #   wave 2: wq/wk hc1               -> S(1,*)
            #   wave 3: xt q2, q3, wp           -> tb2/3 chains, proj
            # xt as 8 grouped tiles [P, 4ec, 512cols]: one 0.5 MiB DMA each
            # (few LARGE transfers — per-transfer fixed cost dominates small
            # chunks), wave-ordered by column quarter qi = demand order.
            xtg = [[big.tile([P, 4, TQ], BF16, tag=f"x{qi}g{g}",
                             name=f"x{qi}g{g}") for g in range(2)]
                   for qi in range(4)]
            wq_all = big.tile([P, 2, N_EC, P], BF16, tag="wq", name="wq")
            wk_all = big.tile([P, 2, N_EC, P], BF16, tag="wk", name="wk")
            wv_all = big.tile([P, N_EC, HDL], BF16, tag="wv", name="wv")
            bk_sb = big.tile([P, 2], F32, tag="bk", name="bk")
            ones_sb = big.tile([P, HL, D], BF16, tag="ones", name="ones")
            ones_r = onesv.rearrange("(h d) -> h d", h=HL)
            mask_sb = big.tile([P, 2, TQ], BF16, tag="mask", name="mask")
            wp_all = big.tile([P, 2, E], BF16, tag="wp", name="wp")

            xt_r = xt.rearrange("(g e p) t -> g p e t", g=2, e=4)
            # sync ring: xt ec0-3, quarter (demand) order
            for qi in range(4):
                nc.sync.dma_start(xtg[qi][0],
                                  xt_r[0, :, :, qi * TQ:(qi + 1) * TQ])
            # scalar ring: weights + mask in demand order
            nc.scalar.dma_start(wq_all[:, 0], wqt[:, 0])
            nc.scalar.dma_start(wk_all[:, 0], wkt[:, 0])
            nc.scalar.dma_start(mask_sb, maskd)
            nc.scalar.dma_start(wv_all, wvt)
            nc.scalar.dma_start(wq_all[:, 1], wqt[:, 1])
            nc.scalar.dma_start(wk_all[:, 1], wkt[:, 1])
            nc.scalar.dma_start(wp_all, wpt)
            # gpsimd ring: tiny tensors then xt ec4-7, quarter order
            nc.gpsimd.dma_start(bk_sb, bkv.rearrange("(c p) -> p c", p=P))
            nc.gpsimd.dma_start(
                ones_sb, bass.AP(tensor=onesv.tensor, offset=onesv.offset,
                                 ap=[[0, P]] + list(ones_r.ap)))
            for qi in range(4):
                nc.gpsimd.dma_start(xtg[qi][1],
                                    xt_r[1, :, :, qi * TQ:(qi + 1) * TQ])

            q_sb = [big.tile([P, T], BF16, tag=f"q{hc}", name=f"q{hc}")
                    for hc in range(2)]
            k_sb = [big.tile([P, T], BF16, tag=f"k{hc}", name=f"k{hc}")
                    for hc in range(2)]
            at_sb = [big.tile([P, T], BF16, tag=f"at{hc}", name=f"at{hc}")
                     for hc in range(2)]
            v_sb = [big.tile([P, HL, 2 * D], BF16, tag=f"v{t}", name=f"v{t}")
                    for t in range(N_TC)]

            def w_lhsT(w_all, ec, hc):
                return w_all[:, hc, ec, :]

            def xt_cols(ec, c0, width):
                qi, o = divmod(c0, TQ)
                assert o + width <= TQ
                return xtg[qi][ec // 4][:, ec % 4, o:o + width]

            def qk_readout(hc, wi, tb, ps, eng="vector"):
                dst = (q_sb, k_sb)[wi][hc][:, tb * TQ:(tb + 1) * TQ]
                if eng == "scalar":
                    # ScalarE is idle before the exp stream starts; its
                    # PSUM->SBUF copy is also faster than DVE's 1x mode.
                    if wi == 1:
                        nc.scalar.activation(dst, ps, AF.Identity,
                                             bias=bk_sb[:, hc:hc + 1])
                    else:
                        nc.scalar.activation(dst, ps, AF.Copy)
                elif wi == 0:
                    nc.vector.tensor_copy(dst, ps)
                else:
                    nc.vector.tensor_scalar_add(dst, ps, bk_sb[:, hc:hc + 1])

            def v_readout(t, ps):
                nc.vector.tensor_copy(
                    v_sb[t][:, :, D:2 * D],
                    ps.rearrange("p (h d) -> p h d", h=HL))

            # ---------------- attention pools ------------------------------
            import contextlib
            _ph34 = contextlib.ExitStack()
            stps = _ph34.enter_context(
                tc.tile_pool(name="stps", bufs=2, space="PSUM"))
            accps = _ph34.enter_context(
                tc.tile_pool(name="accps", bufs=1, space="PSUM"))
            mmps = _ph34.enter_context(
                tc.tile_pool(name="mmps", bufs=2, space="PSUM"))

            # the ones half of every v tile is static: fill while DVE idles
            for t in range(N_TC):
                nc.vector.tensor_copy(v_sb[t][:, :, 0:D], ones_sb)
            pt_ref = {}   # (hp, ib) -> list of staged pt tiles per jb

            def S(hp, ib):
                njb = 4 * ib + 4
                for jb in range(njb):
                    idx = jb - 4 * ib
                    dd = idx * JB if idx >= 0 else 0
                    st = stps.tile([P, 2, TQ], F32, tag="st", name="st")
                    pool = ptp4 if jb < 4 else ptp
                    pt = pool.tile([P, 2, TQ], BF16, tag=f"pt{jb}",
                                   name=f"pt{jb}")
                    pt_ref.setdefault((hp, ib), {})[jb] = pt
                    for h in range(2):
                        pr = slice(h * D, (h + 1) * D)
                        nc.tensor.matmul(
                            st[:, h, dd:],
                            lhsT=q_sb[hp][pr, jb * JB:(jb + 1) * JB],
                            rhs=k_sb[hp][pr, ib * TQ + dd:(ib + 1) * TQ],
                            start=True, stop=True)
                    nc.scalar.activation(pt[:, :, dd:], st[:, :, dd:],
                                         AF.Exp, scale=0.125)
                    if idx >= 0:
                        nc.vector.tensor_mul(pt[:, :, dd:], pt[:, :, dd:],
                                             mask_sb[:, :, :TQ - dd])

            def PV(hp, ib, sliced=False):
                njb = 4 * ib + 4
                accs = [accps.tile([2 * D, TQ], F32, tag=f"acc{h}",
                                   name=f"acc{h}") for h in range(2)]
                for jb in range(njb):
                    idx = jb - 4 * ib
                    dd = idx * JB if idx >= 0 else 0
                    pt = pt_ref[(hp, ib)][jb]
                    for h in range(2):
                        nc.tensor.matmul(
                            accs[h][:, dd:],
                            lhsT=v_sb[jb][:, 2 * hp + h, :],
                            rhs=pt[:, h, dd:],
                            start=(jb == 0), stop=(jb == njb - 1))
                # readout: reciprocal of the ones-row sums, normalize.
                # sliced=True emits per-128-col chunks so the dependent proj
                # t-tiles can start as soon as their chunk is normalized.
                cws = [(c, JB) for c in range(0, TQ, JB)] if sliced \
                    else [(0, TQ)]
                for c0, cw in cws:
                    for h in range(2):
                        rec64 = work.tile([D, TQ], F32, tag="rec64",
                                          name="rec64")
                        nc.vector.reciprocal_approx_fast(
                            rec64[:, 0:cw], accs[h][0:D, c0:c0 + cw])
                        nc.vector.tensor_mul(
                            at_sb[hp][h * D:(h + 1) * D,
                                      ib * TQ + c0:ib * TQ + c0 + cw],
                            accs[h][D:2 * D, c0:c0 + cw], rec64[:, 0:cw])

            def qk_chain(hc, wi, tb, eng="vector"):
                ps = mmps.tile([P, TQ], F32, tag="mm", name="mm")
                for ec in range(N_EC):
                    nc.tensor.matmul(
                        ps, lhsT=w_lhsT((wq_all, wk_all)[wi], ec, hc),
                        rhs=xt_cols(ec, tb * TQ, TQ),
                        start=(ec == 0), stop=(ec == N_EC - 1))
                qk_readout(hc, wi, tb, ps, eng=eng)

            def v_chain(t):
                ps = mmps.tile([P, HDL], F32, tag="mm", name="mm")
                for ec in range(N_EC):
                    nc.tensor.matmul(
                        ps, lhsT=xt_cols(ec, t * P, P),
                        rhs=wv_all[:, ec, :],
                        start=(ec == 0), stop=(ec == N_EC - 1))
                v_readout(t, ps)

            def proj_t(t, eng="vector"):
                ot = outp.tile([P, E], BF16, tag="ot", name="ot")
                for eb in range(2):
                    ps = mmps.tile([P, TQ], F32, tag="mm", name="mm")
                    for hc in range(2):
                        nc.tensor.matmul(
                            ps, lhsT=at_sb[hc][:, t * P:(t + 1) * P],
                            rhs=wp_all[:, hc, eb * TQ:(eb + 1) * TQ],
                            start=(hc == 0), stop=(hc == 1))
                    if eng == "scalar":
                        # ScalarE is idle once the exp stream has drained
                        nc.scalar.activation(ot[:, eb * TQ:(eb + 1) * TQ],
                                             ps, AF.Copy)
                    else:
                        nc.vector.tensor_copy(ot[:, eb * TQ:(eb + 1) * TQ],
                                              ps)
                nc.sync.dma_start(out[t * P:(t + 1) * P, :], ot)

            # ---------------- orchestration --------------------------------
            # All S batches are pulled as far forward as data + pt-slot
            # rotation (bufs=3) allow, so the ScalarE exp stream runs
            # compact and early; PV batches, remaining q/k chains, v waves
            # and the projection are the PE back-fill.
            qk_chain(0, 0, 0, eng="scalar"); qk_chain(0, 1, 0, eng="scalar")
            S(0, 0)
            qk_chain(0, 0, 1); qk_chain(0, 1, 1)
            S(0, 1)
            qk_chain(1, 0, 0); qk_chain(1, 1, 0)
            S(1, 0)
            qk_chain(1, 0, 1); qk_chain(1, 1, 1)
            S(1, 1)
            # the ones half of every v tile is static: one fill pass
            for t in range(N_TC):
                nc.vector.tensor_copy(v_sb[t][:, :, 0:D], ones_sb)
            for t in range(0, 4):
                v_chain(t)
            PV(0, 0)
            qk_chain(0, 0, 2); qk_chain(0, 1, 2)
            S(0, 2)
            for t in range(4, 8):
                v_chain(t)
            PV(0, 1)
            qk_chain(1, 0, 2); qk_chain(1, 1, 2)
            S(1, 2)
            PV(1, 0)
            qk_chain(0, 0, 3); qk_chain(0, 1, 3)
            PV(1, 1)
            S(0, 3)
            for t in range(8, 12):
                v_chain(t)
            PV(0, 2)
            qk_chain(1, 0, 3); qk_chain(1, 1, 3)
            S(1, 3)
            for t in range(12, 16):
                v_chain(t)
            PV(1, 2)
            for t in range(0, 4):
                proj_t(t)
            PV(0, 3)
            for t in range(4, 8):
                proj_t(t)
            PV(1, 3)
            for t in range(8, 12):
                proj_t(t, eng="scalar")
            for t in range(12, 16):
                proj_t(t, eng="scalar")
            _ph34.close()

    nc.compile()
    return nc


def _make_mask():
    # M0[j, c] = (j <= c): the block-diagonal tile idx multiplies its
    # pt[:, :, idx*128:] by the [0:512-idx*128] column prefix of this one
    # pattern (duplicated along axis 1 for the two packed heads).
    jj = np.arange(JB)[:, None]
    cc = np.arange(TQ)[None, :]
    m0 = (jj <= cc).astype(np.float32)
    return np.ascontiguousarray(
        np.broadcast_to(m0[:, None, :], (JB, 2, TQ))).astype(ml_dtypes.bfloat16)


_NC = None


def _get_nc():
    global _NC
    if _NC is None:
        _NC = _build_nc()
    return _NC


def _warr(w):
    """W slice [HDL, E] -> hc-major SBUF layout [P, 2, N_EC, P]: element
    (p, hc, c, d) = W.T[c*P + p, hc*P + d]."""
    return np.ascontiguousarray(
        w.T.reshape(N_EC, P, 2, P).transpose(1, 2, 0, 3)
    ).astype(ml_dtypes.bfloat16)


def _warr_v(w):
    """W slice [HDL, E] -> SBUF layout [P, N_EC, HDL]: element (p, c, f) =
    W.T[c*P + p, f]."""
    return np.ascontiguousarray(
        w.T.reshape(N_EC, P, HDL).transpose(1, 0, 2)).astype(ml_dtypes.bfloat16)


def kernel(x, Wq, bq, Wk, bk, Wv, bv, Wp, bp, **_run_kwargs):
    x = np.asarray(x, dtype=np.float32)
    Wq = np.asarray(Wq, dtype=np.float32)
    Wk = np.asarray(Wk, dtype=np.float32)
    Wv = np.asarray(Wv, dtype=np.float32)
    Wp = np.asarray(Wp, dtype=np.float32)
    bk = np.asarray(bk, dtype=np.float32)
    bv = np.asarray(bv, dtype=np.float32)
    bp = np.asarray(bp, dtype=np.float32)

    mask = _make_mask()

    in_maps = []
    for c in range(NCORES):
        b, hg = divmod(c, GROUPS)
        hsl = slice(HDL * hg, HDL * (hg + 1))
        in_maps.append({
            "xt": np.ascontiguousarray(x[b].T).astype(ml_dtypes.bfloat16),
            "wqt": _warr(Wq[hsl]),
            "wkt": _warr(Wk[hsl]),
            "wvt": _warr_v(Wv[hsl]),
            "wpt": np.ascontiguousarray(
                Wp[:, hsl].T.reshape(2, P, E).transpose(1, 0, 2)
            ).astype(ml_dtypes.bfloat16),
            "bkv": np.ascontiguousarray(bk[hsl]),
            "mask": mask,
            "onesv": np.ones(HDL, dtype=ml_dtypes.bfloat16),
        })

    nc = _get_nc()
    try:
        res = run_bass_kernel_spmd(nc, in_maps, core_ids=list(range(NCORES)),
                                   **_run_kwargs)
    except Exception:
        # transient device hiccups (e.g. NRT_EXEC_UNIT_UNRECOVERABLE) have
        # been observed to clear on retry
        import time
        time.sleep(2.0)
        res = run_bass_kernel_spmd(nc, in_maps, core_ids=list(range(NCORES)),
                                   **_run_kwargs)
    outs = [r["out"].astype(np.float32) for r in res.results]
    # softmax rows sum to 1 -> v/proj biases are a rank-1 host-side add
    bvp = (bv @ Wp.T + bp).astype(np.float32)
    y = np.stack([
        outs[0] + outs[1] + outs[2] + outs[3] + bvp,
        outs[4] + outs[5] + outs[6] + outs[7] + bvp,
    ]).astype(np.float32)
    if _run_kwargs:
        return y, res
    return y


# revision 50
# speedup vs baseline: 1.0119x; 1.0119x over previous
"""Causal attention (B=2, T=2048, E=1024, H=16, D=64) on 8 TRN2 NeuronCores.

Sharding: core c handles batch b = c//4 and local head group hg = c%4
(4 heads, 256 head-dims).  Data parallel over batch, tensor parallel over
heads; the output projection is row-parallel, so each core returns a
partial [T, E] output and the host sums the 4 partials per batch.

Bias algebra (host-side): softmax is over the query axis j, so the
q-bias terms k_i.bq + bk.bq are constant per softmax row and cancel;
only bk survives (applied to k on device).  Softmax rows sum to 1, so
the v-bias contributes exactly bv @ Wp.T to the output; together with
bp it is added on the HOST after the partial-sum gather.  The device
computes pure matmuls + one bk add.

Device plan (per core, all-bf16 matmuls with fp32 PSUM accumulation):
  DMA: three rings (sync HWDGE, scalar HWDGE, gpsimd SWDGE) stream
    concurrently from shared HBM in demand order.  xt is loaded as 8
    grouped 0.5 MiB tiles [P, 4ec, 512cols] (column-quarter major =
    demand order); q/k weights are hc-major so each head-chunk is one
    contiguous 0.25 MiB transfer.  The first exp batch transitively
    needs only {wq/wk hc0, xt quarter 0} — those lead their rings.
  q/k/v production: 8-matmul full-E accumulation chains through a
    2-slot PSUM pool; the first two chain readouts run on the
    still-idle ScalarE (Copy / Identity+bk-bias activations), later
    ones on VectorE.  v tiles are [ones(64) | x@Wv (64)] per head so
    the softmax denominator falls out of the PV matmul.
  attention: decoupled S/PV batches per (head-pair, i-block).
    S(p,ib): per j-tile, 2 concurrent K=64 score MMs (row groups 0-63 /
    64-127), exp on ScalarE (scale=1/8, no max subtraction — scores are
    ~N(0,1)) into SBUF-staged pt tiles, single mask multiply over both
    heads on block-diagonal tiles (one [128,2,512] lower-tri pattern;
    diagonal tile idx reads the [0:512-128*idx] column prefix).
    All S batches are pulled as far forward as data and pt-slot
    rotation (3 slots per j-tile tag) allow, so the ScalarE exp
    stream — the attention pacer — runs compact and early.  PV(p,ib) accumulates v over j from the staged
    pt tiles (never waiting on its own tile's exp); PV, the remaining
    q/k chains, v waves and the projection are the PE back-fill that
    keeps HAM warm.
  proj: per t-tile, 2 accumulating MMs + PSUM->SBUF bf16 copy (Vector
    early, ScalarE for the last tiles once the exp stream has
    drained), out DMA on the sync ring.
"""

import ml_dtypes
import numpy as np

import concourse.bass as bass
import concourse.tile as tile
from concourse import bacc, mybir
from concourse.bass_utils import run_bass_kernel_spmd

B, T, E = 2, 2048, 1024
H, D = 16, 64
NCORES = 8
GROUPS = 4              # cores per batch (tensor parallel over heads)
HL = H // GROUPS        # 4 local heads per core
HDL = HL * D            # 256 local head dims
P = 128
TQ = 512                # i-block (free dim of score tiles)
JB = 128                # j-block (partition dim of score tiles)
N_TB = T // TQ          # 4
N_EC = E // P           # 8
N_TC = T // P           # 16
HALF = T // 2           # 1024 (xt half-tile width)

F32 = mybir.dt.float32
BF16 = mybir.dt.bfloat16
AF = mybir.ActivationFunctionType


def _build_nc():
    nc = bacc.Bacc("TRN2", target_bir_lowering=False, debug=False)
    xt = nc.dram_tensor("xt", [E, T], BF16, kind="ExternalInput").ap()
    # hc-major q/k weight layout: [:, hc] is one contiguous 0.25 MiB DMA
    wqt = nc.dram_tensor("wqt", [P, 2, N_EC, P], BF16, kind="ExternalInput").ap()
    wkt = nc.dram_tensor("wkt", [P, 2, N_EC, P], BF16, kind="ExternalInput").ap()
    wvt = nc.dram_tensor("wvt", [P, N_EC, HDL], BF16, kind="ExternalInput").ap()
    wpt = nc.dram_tensor("wpt", [P, 2, E], BF16, kind="ExternalInput").ap()
    bkv = nc.dram_tensor("bkv", [HDL], F32, kind="ExternalInput").ap()
    maskd = nc.dram_tensor("mask", [JB, 2, TQ], BF16,
                           kind="ExternalInput").ap()
    onesv = nc.dram_tensor("onesv", [HDL], BF16, kind="ExternalInput").ap()
    out = nc.dram_tensor("out", [T, E], BF16, kind="ExternalOutput").ap()

    with tile.TileContext(nc) as tc:
        with (
            tc.tile_pool(name="big", bufs=1) as big,
            tc.tile_pool(name="ptp", bufs=3) as ptp,
            tc.tile_pool(name="work", bufs=2) as work,
            tc.tile_pool(name="outp", bufs=3) as outp,
        ):
            # ---------------- DMA posts (order = transfer order per ring) ---
            # The first exp transitively needs only q/k(hc0) weights + the
            # tb0 columns of xt, so xt is posted as [P,512] quarters and
            # wq/wk as hc slices, interleaved across all three rings in
            # demand order (rings share HBM bandwidth; per-ring streams are
            # serial, so each demand wave is split across rings):
            #   wave 0: wq/wk hc0, xt q0 (tb0)  -> S(0,0)
            #   wave 1: wv, xt q1 (tb1)         -> v chains, S(0,1)
            #   wave 2: wq/wk hc1               -> S(1,*)
            #   wave 3: xt q2, q3, wp           -> tb2/3 chains, proj
            # xt as 8 grouped tiles [P, 4ec, 512cols]: one 0.5 MiB DMA each
            # (few LARGE transfers — per-transfer fixed cost dominates small
            # chunks), wave-ordered by column quarter qi = demand order.
            xtg = [[big.tile([P, 4, TQ], BF16, tag=f"x{qi}g{g}",
                             name=f"x{qi}g{g}") for g in range(2)]
                   for qi in range(4)]
            wq_all = big.tile([P, 2, N_EC, P], BF16, tag="wq", name="wq")
            wk_all = big.tile([P, 2, N_EC, P], BF16, tag="wk", name="wk")
            wv_all = big.tile([P, N_EC, HDL], BF16, tag="wv", name="wv")
            bk_sb = big.tile([P, 2], F32, tag="bk", name="bk")
            ones_sb = big.tile([P, HL, D], BF16, tag="ones", name="ones")
            ones_r = onesv.rearrange("(h d) -> h d", h=HL)
            mask_sb = big.tile([P, 2, TQ], BF16, tag="mask", name="mask")
            wp_all = big.tile([P, 2, E], BF16, tag="wp", name="wp")

            xt_r = xt.rearrange("(g e p) t -> g p e t", g=2, e=4)
            # sync ring: xt ec0-3, quarter (demand) order
            for qi in range(4):
                nc.sync.dma_start(xtg[qi][0],
                                  xt_r[0, :, :, qi * TQ:(qi + 1) * TQ])
            # scalar ring: weights + mask in demand order
            nc.scalar.dma_start(wq_all[:, 0], wqt[:, 0])
            nc.scalar.dma_start(wk_all[:, 0], wkt[:, 0])
            nc.scalar.dma_start(mask_sb, maskd)
            nc.scalar.dma_start(wv_all, wvt)
            nc.scalar.dma_start(wq_all[:, 1], wqt[:, 1])
            nc.scalar.dma_start(wk_all[:, 1], wkt[:, 1])
            nc.scalar.dma_start(wp_all, wpt)
            # gpsimd ring: tiny tensors then xt ec4-7, quarter order
            nc.gpsimd.dma_start(bk_sb, bkv.rearrange("(c p) -> p c", p=P))
            nc.gpsimd.dma_start(
                ones_sb, bass.AP(tensor=onesv.tensor, offset=onesv.offset,
                                 ap=[[0, P]] + list(ones_r.ap)))
            for qi in range(4):
                nc.gpsimd.dma_start(xtg[qi][1],
                                    xt_r[1, :, :, qi * TQ:(qi + 1) * TQ])

            q_sb = [big.tile([P, T], BF16, tag=f"q{hc}", name=f"q{hc}")
                    for hc in range(2)]
            k_sb = [big.tile([P, T], BF16, tag=f"k{hc}", name=f"k{hc}")
                    for hc in range(2)]
            at_sb = [big.tile([P, T], BF16, tag=f"at{hc}", name=f"at{hc}")
                     for hc in range(2)]
            v_sb = [big.tile([P, HL, 2 * D], BF16, tag=f"v{t}", name=f"v{t}")
                    for t in range(N_TC)]

            def w_lhsT(w_all, ec, hc):
                return w_all[:, hc, ec, :]

            def xt_cols(ec, c0, width):
                qi, o = divmod(c0, TQ)
                assert o + width <= TQ
                return xtg[qi][ec // 4][:, ec % 4, o:o + width]

            def qk_readout(hc, wi, tb, ps, eng="vector"):
                dst = (q_sb, k_sb)[wi][hc][:, tb * TQ:(tb + 1) * TQ]
                if eng == "scalar":
                    # ScalarE is idle before the exp stream starts; its
                    # PSUM->SBUF copy is also faster than DVE's 1x mode.
                    if wi == 1:
                        nc.scalar.activation(dst, ps, AF.Identity,
                                             bias=bk_sb[:, hc:hc + 1])
                    else:
                        nc.scalar.activation(dst, ps, AF.Copy)
                elif wi == 0:
                    nc.vector.tensor_copy(dst, ps)
                else:
                    nc.vector.tensor_scalar_add(dst, ps, bk_sb[:, hc:hc + 1])

            def v_readout(t, ps):
                nc.vector.tensor_copy(v_sb[t][:, :, 0:D], ones_sb)
                nc.vector.tensor_copy(
                    v_sb[t][:, :, D:2 * D],
                    ps.rearrange("p (h d) -> p h d", h=HL))

            # ---------------- attention pools ------------------------------
            import contextlib
            _ph34 = contextlib.ExitStack()
            stps = _ph34.enter_context(
                tc.tile_pool(name="stps", bufs=2, space="PSUM"))
            accps = _ph34.enter_context(
                tc.tile_pool(name="accps", bufs=1, space="PSUM"))
            mmps = _ph34.enter_context(
                tc.tile_pool(name="mmps", bufs=2, space="PSUM"))

            # the ones half of every v tile is static: fill while DVE idles
            for t in range(N_TC):
                nc.vector.tensor_copy(v_sb[t][:, :, 0:D], ones_sb)
            pt_ref = {}   # (hp, ib) -> list of staged pt tiles per jb

            def S(hp, ib):
                njb = 4 * ib + 4
                for jb in range(njb):
                    idx = jb - 4 * ib
                    dd = idx * JB if idx >= 0 else 0
                    st = stps.tile([P, 2, TQ], F32, tag="st", name="st")
                    pt = ptp.tile([P, 2, TQ], BF16, tag=f"pt{jb}",
                                  name=f"pt{jb}")
                    pt_ref.setdefault((hp, ib), {})[jb] = pt
                    for h in range(2):
                        pr = slice(h * D, (h + 1) * D)
                        nc.tensor.matmul(
                            st[:, h, dd:],
                            lhsT=q_sb[hp][pr, jb * JB:(jb + 1) * JB],
                            rhs=k_sb[hp][pr, ib * TQ + dd:(ib + 1) * TQ],
                            start=True, stop=True)
                    nc.scalar.activation(pt[:, :, dd:], st[:, :, dd:],
                                         AF.Exp, scale=0.125)
                    if idx >= 0:
                        nc.vector.tensor_mul(pt[:, :, dd:], pt[:, :, dd:],
                                             mask_sb[:, :, :TQ - dd])

            def PV(hp, ib, sliced=False):
                njb = 4 * ib + 4
                accs = [accps.tile([2 * D, TQ], F32, tag=f"acc{h}",
                                   name=f"acc{h}") for h in range(2)]
                for jb in range(njb):
                    idx = jb - 4 * ib
                    dd = idx * JB if idx >= 0 else 0
                    pt = pt_ref[(hp, ib)][jb]
                    for h in range(2):
                        nc.tensor.matmul(
                            accs[h][:, dd:],
                            lhsT=v_sb[jb][:, 2 * hp + h, :],
                            rhs=pt[:, h, dd:],
                            start=(jb == 0), stop=(jb == njb - 1))
                # readout: reciprocal of the ones-row sums, normalize.
                # sliced=True emits per-128-col chunks so the dependent proj
                # t-tiles can start as soon as their chunk is normalized.
                cws = [(c, JB) for c in range(0, TQ, JB)] if sliced \
                    else [(0, TQ)]
                for c0, cw in cws:
                    for h in range(2):
                        rec64 = work.tile([D, TQ], F32, tag="rec64",
                                          name="rec64")
                        nc.vector.reciprocal_approx_fast(
                            rec64[:, 0:cw], accs[h][0:D, c0:c0 + cw])
                        nc.vector.tensor_mul(
                            at_sb[hp][h * D:(h + 1) * D,
                                      ib * TQ + c0:ib * TQ + c0 + cw],
                            accs[h][D:2 * D, c0:c0 + cw], rec64[:, 0:cw])

            def qk_chain(hc, wi, tb, eng="vector"):
                ps = mmps.tile([P, TQ], F32, tag="mm", name="mm")
                for ec in range(N_EC):
                    nc.tensor.matmul(
                        ps, lhsT=w_lhsT((wq_all, wk_all)[wi], ec, hc),
                        rhs=xt_cols(ec, tb * TQ, TQ),
                        start=(ec == 0), stop=(ec == N_EC - 1))
                qk_readout(hc, wi, tb, ps, eng=eng)

            def v_chain(t):
                ps = mmps.tile([P, HDL], F32, tag="mm", name="mm")
                for ec in range(N_EC):
                    nc.tensor.matmul(
                        ps, lhsT=xt_cols(ec, t * P, P),
                        rhs=wv_all[:, ec, :],
                        start=(ec == 0), stop=(ec == N_EC - 1))
                v_readout(t, ps)

            def proj_t(t, eng="vector"):
                ot = outp.tile([P, E], BF16, tag="ot", name="ot")
                for eb in range(2):
                    ps = mmps.tile([P, TQ], F32, tag="mm", name="mm")
                    for hc in range(2):
                        nc.tensor.matmul(
                            ps, lhsT=at_sb[hc][:, t * P:(t + 1) * P],
                            rhs=wp_all[:, hc, eb * TQ:(eb + 1) * TQ],
                            start=(hc == 0), stop=(hc == 1))
                    if eng == "scalar":
                        # ScalarE is idle once the exp stream has drained
                        nc.scalar.activation(ot[:, eb * TQ:(eb + 1) * TQ],
                                             ps, AF.Copy)
                    else:
                        nc.vector.tensor_copy(ot[:, eb * TQ:(eb + 1) * TQ],
                                              ps)
                nc.sync.dma_start(out[t * P:(t + 1) * P, :], ot)

            # ---------------- orchestration --------------------------------
            # All S batches are pulled as far forward as data + pt-slot
            # rotation (bufs=3) allow, so the ScalarE exp stream runs
            # compact and early; PV batches, remaining q/k chains, v waves
            # and the projection are the PE back-fill.
            qk_chain(0, 0, 0, eng="scalar"); qk_chain(0, 1, 0, eng="scalar")
            S(0, 0)
            qk_chain(0, 0, 1); qk_chain(0, 1, 1)
            S(0, 1)
            qk_chain(1, 0, 0); qk_chain(1, 1, 0)
            S(1, 0)
            for t in range(0, 4):
                v_chain(t)
            PV(0, 0)
            qk_chain(1, 0, 1); qk_chain(1, 1, 1)
            S(1, 1)
            for t in range(4, 8):
                v_chain(t)
            PV(0, 1)
            qk_chain(0, 0, 2); qk_chain(0, 1, 2)
            S(0, 2)
            PV(1, 0)
            qk_chain(1, 0, 2); qk_chain(1, 1, 2)
            S(1, 2)
            PV(1, 1)
            qk_chain(0, 0, 3); qk_chain(0, 1, 3)
            S(0, 3)
            for t in range(8, 12):
                v_chain(t)
            PV(0, 2)
            qk_chain(1, 0, 3); qk_chain(1, 1, 3)
            S(1, 3)
            for t in range(12, 16):
                v_chain(t)
            PV(1, 2)
            for t in range(0, 4):
                proj_t(t)
            PV(0, 3)
            for t in range(4, 8):
                proj_t(t)
            PV(1, 3)
            for t in range(8, 12):
                proj_t(t, eng="scalar")
            for t in range(12, 16):
                proj_t(t, eng="scalar")
            _ph34.close()

    nc.compile()
    return nc


def _make_mask():
    # M0[j, c] = (j <= c): the block-diagonal tile idx multiplies its
    # pt[:, :, idx*128:] by the [0:512-idx*128] column prefix of this one
    # pattern (duplicated along axis 1 for the two packed heads).
    jj = np.arange(JB)[:, None]
    cc = np.arange(TQ)[None, :]
    m0 = (jj <= cc).astype(np.float32)
    return np.ascontiguousarray(
        np.broadcast_to(m0[:, None, :], (JB, 2, TQ))).astype(ml_dtypes.bfloat16)


_NC = None


def _get_nc():
    global _NC
    if _NC is None:
        _NC = _build_nc()
    return _NC


def _warr(w):
    """W slice [HDL, E] -> hc-major SBUF layout [P, 2, N_EC, P]: element
    (p, hc, c, d) = W.T[c*P + p, hc*P + d]."""
    return np.ascontiguousarray(
        w.T.reshape(N_EC, P, 2, P).transpose(1, 2, 0, 3)
    ).astype(ml_dtypes.bfloat16)


def _warr_v(w):
    """W slice [HDL, E] -> SBUF layout [P, N_EC, HDL]: element (p, c, f) =
    W.T[c*P + p, f]."""
    return np.ascontiguousarray(
        w.T.reshape(N_EC, P, HDL).transpose(1, 0, 2)).astype(ml_dtypes.bfloat16)


def kernel(x, Wq, bq, Wk, bk, Wv, bv, Wp, bp, **_run_kwargs):
    x = np.asarray(x, dtype=np.float32)
    Wq = np.asarray(Wq, dtype=np.float32)
    Wk = np.asarray(Wk, dtype=np.float32)
    Wv = np.asarray(Wv, dtype=np.float32)
    Wp = np.asarray(Wp, dtype=np.float32)
    bk = np.asarray(bk, dtype=np.float32)
    bv = np.asarray(bv, dtype=np.float32)
    bp = np.asarray(bp, dtype=np.float32)

    mask = _make_mask()

    in_maps = []
    for c in range(NCORES):
        b, hg = divmod(c, GROUPS)
        hsl = slice(HDL * hg, HDL * (hg + 1))
        in_maps.append({
            "xt": np.ascontiguousarray(x[b].T).astype(ml_dtypes.bfloat16),
            "wqt": _warr(Wq[hsl]),
            "wkt": _warr(Wk[hsl]),
            "wvt": _warr_v(Wv[hsl]),
            "wpt": np.ascontiguousarray(
                Wp[:, hsl].T.reshape(2, P, E).transpose(1, 0, 2)
            ).astype(ml_dtypes.bfloat16),
            "bkv": np.ascontiguousarray(bk[hsl]),
            "mask": mask,
            "onesv": np.ones(HDL, dtype=ml_dtypes.bfloat16),
        })

    nc = _get_nc()
    try:
        res = run_bass_kernel_spmd(nc, in_maps, core_ids=list(range(NCORES)),
                                   **_run_kwargs)
    except Exception:
        # transient device hiccups (e.g. NRT_EXEC_UNIT_UNRECOVERABLE) have
        # been observed to clear on retry
        import time
        time.sleep(2.0)
        res = run_bass_kernel_spmd(nc, in_maps, core_ids=list(range(NCORES)),
                                   **_run_kwargs)
    outs = [r["out"].astype(np.float32) for r in res.results]
    # softmax rows sum to 1 -> v/proj biases are a rank-1 host-side add
    bvp = (bv @ Wp.T + bp).astype(np.float32)
    y = np.stack([
        outs[0] + outs[1] + outs[2] + outs[3] + bvp,
        outs[4] + outs[5] + outs[6] + outs[7] + bvp,
    ]).astype(np.float32)
    if _run_kwargs:
        return y, res
    return y
